# revision 13
# baseline (speedup 1.0000x reference)
"""GNN message-passing kernel for 8 Trainium2 NeuronCores (v2-safe).

Math (per reference):
  h   = relu(ef @ W1 + b1)                      [E, H]
  K   = (h @ W2 + b2).reshape(E, G, L)          per-edge [G, L] kernels
  t   = einsum('bnl,ne->bel', x, inc)           gather nodes->edges
  y   = einsum('egl,bel->beg', K, t)            per-edge matvec
  out = relu(einsum('ne,beg->bng', inc, y) + b_gc).reshape(B, N*G)

Distribution: shard E across the 8 cores (2000 edges each, padded 2048).
Partial sums reduced on host.

vs baseline: mlp2 and gather matmuls use a 256-wide moving dim (half the
PE instruction count), draining each [128, 256] PSUM tile into two
128-edge chunk tiles; the per-edge matvec and PE transposes keep the
proven baseline structure per 128-edge chunk. Phase 2 is restructured:
Y tiles are the stationary operand (128-col bf16 -> fast weight load),
incT[:, 0:500] is the moving operand, all 8 PSUM banks accumulate, and
the output is written as [B*G, 500] f32 which the host transposes.
"""

import numpy as np
import ml_dtypes

import concourse.bass as bass
from concourse import bacc
import concourse.mybir as mybir
import concourse.tile as tile
from concourse.bass_utils import run_bass_kernel_spmd
from concourse.masks import make_identity

B, N, E, L, G, F, H = 64, 500, 16000, 64, 64, 8, 128
NCORES = 8
ELR = E // NCORES       # 2000 real edges per core
EL = 2048               # padded; pad edges have zero incidence columns
SC = 256                # super-chunk (mlp2/gather moving dim)
NSC = EL // SC          # 8
EC = 128                # chunk (matvec/transpose/y_d granularity)
NCH = EL // EC          # 16
NP = 125                # nodes per n-chunk (500 = 4*125)
NQ = 4
BG = B * G              # 4096
F32 = mybir.dt.float32
BF16 = mybir.dt.bfloat16
RELU = mybir.ActivationFunctionType.Relu
IDENT = mybir.ActivationFunctionType.Identity

_CACHE = {}
last_results = None


def _build():
    nc = bacc.Bacc("TRN2", target_bir_lowering=False)
    xT_d = nc.declare_dram_parameter("xT", [N, B * L], BF16, isOutput=False)
    inc_d = nc.declare_dram_parameter("inc", [N, EL], BF16, isOutput=False)
    incT_d = nc.declare_dram_parameter("incT", [EL, N], BF16, isOutput=False)
    efT_d = nc.declare_dram_parameter("efT", [F, EL], BF16, isOutput=False)
    W1_d = nc.declare_dram_parameter("W1", [F, H], BF16, isOutput=False)
    b1_d = nc.declare_dram_parameter("b1", [H, 1], F32, isOutput=False)
    W2_d = nc.declare_dram_parameter("W2", [H, G * L], BF16, isOutput=False)
    b2T_d = nc.declare_dram_parameter("b2T", [H, G * L // H], F32, isOutput=False)
    out_d = nc.declare_dram_parameter("out", [BG, N], F32, isOutput=True)
    y_d = nc.dram_tensor("Ystage", [EL, BG], BF16)

    with tile.TileContext(nc) as tc, tc.tile_pool(name="const", bufs=1) as cpool:
        with tc.tile_pool(name="h_ps", bufs=2, space="PSUM") as hps:
            # ---- persistent tiles ----
            xT_sb = cpool.tile([NP, NQ, B * L], BF16)       # 32KB/part
            nc.sync.dma_start(
                out=xT_sb[:, :, :],
                in_=xT_d[:, :].rearrange("(q n) c -> n q c", q=NQ),
            )
            W1_sb = cpool.tile([F, H], BF16)
            nc.sync.dma_start(out=W1_sb[:, :], in_=W1_d[:, :])
            b1_sb = cpool.tile([H, 1], F32)
            nc.sync.dma_start(out=b1_sb[:, :], in_=b1_d[:, :])
            W2_sb = cpool.tile([H, G * L], BF16)            # 8KB/part
            nc.sync.dma_start(out=W2_sb[:, :], in_=W2_d[:, :])
            b2T_sb = cpool.tile([H, G * L // H], F32)
            nc.sync.dma_start(out=b2T_sb[:, :], in_=b2T_d[:, :])
            efT_sb = cpool.tile([F, EL], BF16)
            nc.sync.dma_start(out=efT_sb[:, :], in_=efT_d[:, :])
            ident = cpool.tile([L, L], BF16)
            make_identity(nc, ident[:, :])
            hT_sb = cpool.tile([H, EL], BF16)               # 4KB/part
            # block-diagonal per-edge-pair kernels: pair (pi, pi+128),
            # kbd[l, pi, g] holds e0's [L,G] in the top-left 64x64 block
            # and e1's in the bottom-right; off-diagonal blocks stay 0.
            kbds = [cpool.tile([2 * L, EC, 2 * G], BF16, name=f"kbd{i}")
                    for i in range(2)]
            for kbd in kbds:
                nc.gpsimd.memset(kbd[0:L, :, G:2 * G], 0.0)
                nc.gpsimd.memset(kbd[L:2 * L, :, 0:G], 0.0)

            # ---- mlp1: hT = relu(W1.T @ efT + b1) ----
            for c in range(4):
                ph = hps.tile([H, 512], F32)
                nc.tensor.matmul(
                    ph[:, :], lhsT=W1_sb[:, :],
                    rhs=efT_sb[:, c * 512:(c + 1) * 512],
                    start=True, stop=True,
                )
                nc.scalar.activation(
                    hT_sb[:, c * 512:(c + 1) * 512], ph[:, :], RELU,
                    bias=b1_sb[:, 0:1],
                )

        # ---- phase 1 ----
        with (
            tc.tile_pool(name="stream", bufs=2) as spool,
            tc.tile_pool(name="tt", bufs=2) as ttpool,
            tc.tile_pool(name="ycp", bufs=1) as ycppool,
            tc.tile_pool(name="yfin", bufs=2) as yfpool,
            tc.tile_pool(name="mlp2_ps", bufs=2, space="PSUM") as mps,
            tc.tile_pool(name="gat_ps", bufs=2, space="PSUM") as gps,
            tc.tile_pool(name="mv_ps", bufs=2, space="PSUM") as vps,
            tc.tile_pool(name="tr_ps", bufs=2, space="PSUM") as tps,
        ):
            for sc in range(NSC):
                e0 = sc * SC
                kbd = kbds[sc % 2]
                # ---- mlp2 at N=256 -> block-diag kbd[(l2), pi, (g2)] ----
                for mc in range(32):
                    pm = mps.tile([H, SC], F32, tag="m2")
                    nc.tensor.matmul(
                        pm[:, :], lhsT=W2_sb[:, mc * H:(mc + 1) * H],
                        rhs=hT_sb[:, e0:e0 + SC], start=True, stop=True,
                    )
                    for par in (0, 1):
                        g = 2 * mc + par
                        bias = b2T_sb[par * 64:(par + 1) * 64, mc:mc + 1]
                        for hh in (0, 1):
                            src = pm[par * 64:(par + 1) * 64,
                                     hh * EC:(hh + 1) * EC]
                            dst = kbd[hh * L:(hh + 1) * L, :, hh * G + g]
                            if (mc + hh) % 2 == 0:
                                nc.scalar.activation(dst, src, IDENT,
                                                     bias=bias)
                            else:
                                nc.vector.tensor_scalar_add(dst, src, bias)

                # ---- gather at N=256 -> two chunk tiles tT[l, b, 128] ----
                inc_t = spool.tile([NP, NQ, SC], BF16, tag="inc")
                nc.sync.dma_start(
                    out=inc_t[:, :, :],
                    in_=inc_d[:, e0:e0 + SC].rearrange(
                        "(q n) e -> n q e", q=NQ),
                )
                # banded tT2: rows 0-63 = l of edge pi, 64-127 = l of
                # edge pi+128
                tT2 = ttpool.tile([2 * L, B, EC], BF16, tag="tt")
                for bp in range(B // 2):
                    pg = gps.tile([2 * L, SC], F32, tag="g")
                    for q in range(NQ):
                        nc.tensor.matmul(
                            pg[:, :],
                            lhsT=xT_sb[:, q, bp * 128:(bp + 1) * 128],
                            rhs=inc_t[:, q, :],
                            start=(q == 0), stop=(q == NQ - 1),
                        )
                    for par in (0, 1):
                        for hh in (0, 1):
                            src = pg[par * 64:(par + 1) * 64,
                                     hh * EC:(hh + 1) * EC]
                            dst = tT2[hh * L:(hh + 1) * L, 2 * bp + par, :]
                            if (bp + hh) % 2 == 0:
                                nc.scalar.copy(dst, src)
                            else:
                                nc.vector.tensor_copy(dst, src)

                # ---- matvec: 2 edges per MM via block-diag stationary ----
                # ycp[g, hh, j, p, b]: edge e = hh*128 + 4*j + p
                ycp = ycppool.tile([G, 2, EC // 4, 4, B], BF16, tag="ycp")
                for j in range(EC // 4):
                    pv = vps.tile([2 * G, 4, B], F32, tag="mv")
                    for p in range(4):
                        pi = 4 * j + p
                        nc.tensor.matmul(
                            pv[:, p, :], lhsT=kbd[:, pi, :],
                            rhs=tT2[:, :, pi],
                            start=True, stop=True,
                        )
                    for hh in (0, 1):
                        src = pv[hh * G:(hh + 1) * G, :, :]
                        dst = ycp[:, hh, j, :, :]
                        if (j + hh) % 2 == 0:
                            nc.scalar.copy(dst, src)
                        else:
                            nc.vector.tensor_copy(dst, src)

                # ---- per 128-edge chunk: transpose (baseline) ----
                for hh in (0, 1):
                    yfin = yfpool.tile([EC, B, G], BF16, tag="yf")
                    for b8 in range(B // 8):
                        pt = tps.tile([EC, 8, G], BF16, tag="tr")
                        for i in range(8):
                            b = b8 * 8 + i
                            nc.tensor.transpose(
                                pt[:, i, :], ycp[:, hh, :, :, b],
                                ident[:, :],
                            )
                        if b8 % 2 == 0:
                            nc.vector.tensor_copy(
                                yfin[:, b8 * 8:(b8 + 1) * 8, :],
                                pt[:, :, :])
                        else:
                            nc.scalar.copy(
                                yfin[:, b8 * 8:(b8 + 1) * 8, :],
                                pt[:, :, :])
                    nc.sync.dma_start(
                        out=y_d[e0 + hh * EC:e0 + (hh + 1) * EC, :],
                        in_=yfin[:, :, :],
                    )

        # ---- phase 2: scatter, Y stationary (FWL), incT moving ----
        with (
            tc.tile_pool(name="p2c", bufs=1) as p2c,
            tc.tile_pool(name="p2y", bufs=2) as p2y,
            tc.tile_pool(name="p2o", bufs=2) as p2o,
            tc.tile_pool(name="acc_ps", bufs=8, space="PSUM") as aps,
        ):
            incT_sb = p2c.tile([EC, NCH, N], BF16)          # 16KB/part
            nc.sync.dma_start(
                out=incT_sb[:, :, :],
                in_=incT_d[:, :].rearrange("(c e) n -> e c n", c=NCH),
            )
            for bgg in range(BG // 1024):
                paccs = [aps.tile([128, N], F32, tag="acc",
                                  name=f"acc{bgg}_{t}") for t in range(8)]
                for ec in range(NCH):
                    yt = p2y.tile([EC, 1024], BF16, tag="yt")
                    nc.sync.dma_start(
                        out=yt[:, :],
                        in_=y_d[ec * EC:(ec + 1) * EC,
                                bgg * 1024:(bgg + 1) * 1024],
                    )
                    for t in range(8):
                        nc.tensor.matmul(
                            paccs[t][:, :],
                            lhsT=yt[:, t * 128:(t + 1) * 128],
                            rhs=incT_sb[:, ec, :],
                            start=(ec == 0), stop=(ec == NCH - 1),
                        )
                for t in range(8):
                    ot = p2o.tile([128, N], F32, tag="ost",
                                  name=f"ost{bgg}_{t}")
                    if t % 2 == 0:
                        nc.vector.tensor_copy(ot[:, :], paccs[t][:, :])
                    else:
                        nc.scalar.copy(ot[:, :], paccs[t][:, :])
                    nc.sync.dma_start(
                        out=out_d[bgg * 1024 + t * 128:
                                  bgg * 1024 + (t + 1) * 128, :],
                        in_=ot[:, :],
                    )
    nc.compile()
    return nc


def kernel(x, incidence, ef, W1, b1, W2, b2, b_gc):
    global last_results
    x = np.asarray(x, dtype=np.float32)
    incidence = np.asarray(incidence, dtype=np.float32)
    ef = np.asarray(ef, dtype=np.float32)
    W1 = np.asarray(W1, dtype=np.float32)
    b1 = np.asarray(b1, dtype=np.float32)
    W2 = np.asarray(W2, dtype=np.float32)
    b2 = np.asarray(b2, dtype=np.float32)
    b_gc = np.asarray(b_gc, dtype=np.float32)

    if "nc" not in _CACHE:
        _CACHE["nc"] = _build()
    nc = _CACHE["nc"]

    bf = ml_dtypes.bfloat16
    xT = np.ascontiguousarray(
        x.transpose(1, 0, 2).reshape(N, B * L)).astype(bf)
    inc_bf = incidence.astype(bf)
    incT_bf = np.ascontiguousarray(incidence.T).astype(bf)
    efT = np.ascontiguousarray(ef.T).astype(bf)
    b1c = np.ascontiguousarray(b1.reshape(H, 1))
    W2_bf = W2.astype(bf)
    b2T = np.ascontiguousarray(b2.reshape(G * L // H, H).T)

    pad = EL - ELR
    in_maps = []
    for c in range(NCORES):
        es = slice(c * ELR, (c + 1) * ELR)
        in_maps.append({
            "xT": xT,
            "inc": np.ascontiguousarray(
                np.pad(inc_bf[:, es], ((0, 0), (0, pad)))),
            "incT": np.ascontiguousarray(
                np.pad(incT_bf[es, :], ((0, pad), (0, 0)))),
            "efT": np.ascontiguousarray(
                np.pad(efT[:, es], ((0, 0), (0, pad)))),
            "W1": W1.astype(bf), "b1": b1c, "W2": W2_bf, "b2T": b2T,
        })

    import os
    trace = bool(int(os.environ.get("KERNEL_TRACE", "0")))
    last_results = run_bass_kernel_spmd(
        nc, in_maps, list(range(NCORES)), trace=trace)
    partial = np.zeros((BG, N), np.float32)
    for r in last_results.results:
        partial += r["out"]
    # out rows are (b, g) b-major; transpose n/g on host
    out = partial.reshape(B, G, N).transpose(0, 2, 1)
    out = out + b_gc.reshape(1, 1, G)
    out = np.maximum(out, 0.0)
    return out.reshape(B, N * G).astype(np.float32)
